# revision 1
# baseline (speedup 1.0000x reference)
"""Trainium2 Bass kernel for CompressedLinear (VQ codebook linear layer).

Computes: out = x @ W^T + bias, where
  W = (centroids[indices] @ Pi) * row_norms[:, None]

Sharding: out_features (4096) split across 8 cores (512 each); x replicated.
Per-core device pipeline:
  1. Gather yts[j,o] = centroids[idxT[j,o]] via fused custom-DVE ops (2
     codebook entries per instruction, 8 instructions per tile).
  2. W_u^T[i,o] = sum_j Pi[j,i] * yts[j,o] on the PE (bf16, f32 psum).
  3. outT[o,t] = sum_i W_u^T[i,o] * xT[i,t]; then out = rn*acc + bias on DVE.
Host feeds x pre-transposed/bf16-cast (layout prep), Pi in column-stripe
layout, indices transposed; host reassembles the 8 outT shards.
"""

import numpy as np

# Problem geometry (hardcoded per contract)
OUT, IN = 4096, 4096
B, S = 4, 2048
T = B * S          # 8192 tokens
NCORES = 8
P = 128            # partitions

_DVE_OPS = None
_NC_CACHE = {}


def _register_dve_ops():
    """Register the two fused VQ-gather ops in dve_ops.OPS (idempotent)."""
    global _DVE_OPS
    if _DVE_OPS is not None:
        return _DVE_OPS
    import concourse.dve_ops as dvo
    from concourse.dve_spec import Spec, Src0, Src1, C0, C1, C2, One, eq, lower
    from concourse.dve_uop import DveOpSpec

    existing = {op.name: op for op in dvo.OPS}
    if "VQ_PAIR" in existing:
        _DVE_OPS = {k: existing[k] for k in ("VQ_PAIR", "VQ_ACC2")}
        return _DVE_OPS

    ver = "v3"  # TRN2

    def mk(name, spec, rd1):
        opcode = dvo._CUSTOM_DVE_ROW_BASE + len(dvo.OPS)
        dvo._SUB_OPCODE_FOR_NAME[name] = opcode
        s = DveOpSpec(name=name, opcode=opcode, uops=lower(spec, ver=ver), rd1_en=rd1)
        op = dvo.DveOp(name, spec, subdim=False, uops_sha={ver: s.sha(ver)})
        dvo.OPS.append(op)
        dvo.CUSTOM_DVE_SPECS[name] = spec
        return op

    # out = (idx==imm2)*s0 + (idx==imm2+1)*s1
    pair = mk(
        "VQ_PAIR",
        Spec(
            body=eq(Src0, C2) * C0 + eq(Src0, C2 + One) * C1,
            reference=lambda in0, in1, s0, s1, imm2: (
                (in0 == imm2) * s0 + (in0 == imm2 + 1) * s1
            ).astype(np.float32),
        ),
        False,
    )
    # out = acc + (idx==imm2)*s0 + (idx==imm2+1)*s1
    acc = mk(
        "VQ_ACC2",
        Spec(
            body=Src1 + eq(Src0, C2) * C0 + eq(Src0, C2 + One) * C1,
            reference=lambda in0, in1, s0, s1, imm2: (
                in1 + (in0 == imm2) * s0 + (in0 == imm2 + 1) * s1
            ).astype(np.float32),
        ),
        True,
    )
    _DVE_OPS = {"VQ_PAIR": pair, "VQ_ACC2": acc}
    return _DVE_OPS


def build_nc(cvals, in_=IN, t=T, osh=OUT // NCORES, tch=512, igrp=4):
    """Build the SPMD Bass program. cvals: 16 python floats (codebook)."""
    import concourse.bacc as bacc
    import concourse.mybir as mybir
    from concourse.tile import TileContext

    f32 = mybir.dt.float32
    bf16 = mybir.dt.bfloat16

    nj = in_ // P          # j blocks (rows of Pi / x input dim)
    ni = in_ // P          # i blocks (cols of Pi / contraction of main mm)
    nob = osh // P         # output feature blocks per core
    nt = t // tch          # token chunks
    ngrp = ni // igrp      # i-groups for the W^T stage

    nc = bacc.Bacc()
    xT_d = nc.dram_tensor("xT", [in_, t], bf16, kind="ExternalInput")
    piR_d = nc.dram_tensor("PiR", [ni, in_, P], bf16, kind="ExternalInput")
    idxT_d = nc.dram_tensor("idxT", [in_, osh], bf16, kind="ExternalInput")
    rn_d = nc.dram_tensor("rn", [osh], f32, kind="ExternalInput")
    bias_d = nc.dram_tensor("bias", [osh], f32, kind="ExternalInput")
    outT_d = nc.dram_tensor("outT", [osh, t], f32, kind="ExternalOutput")

    with TileContext(nc) as tc:
        with (
            tc.tile_pool(name="constp", bufs=1) as constp,
            tc.tile_pool(name="idxp", bufs=3) as idxp,
            tc.tile_pool(name="ytsp", bufs=1) as ytsp,
            tc.tile_pool(name="pip", bufs=igrp + 2) as pip,
            tc.tile_pool(name="wtp", bufs=1) as wtp,
            tc.tile_pool(name="xtp", bufs=2) as xtp,
            tc.tile_pool(name="outp", bufs=4) as outp,
            tc.tile_pool(name="wpsum", bufs=1, space="PSUM") as wpsum,
            tc.tile_pool(name="mpsum", bufs=2, space="PSUM") as mpsum,
        ):
            rn_sb = constp.tile([P, nob], f32, name="rn_sb")
            nc.sync.dma_start(rn_sb[:], rn_d.rearrange("(b p) -> p b", p=P))
            bias_sb = constp.tile([P, nob], f32, name="bias_sb")
            nc.sync.dma_start(bias_sb[:], bias_d.rearrange("(b p) -> p b", p=P))

            # ---- Stage 1: codebook gather: yts[j][p, o] = centroids[idxT] --
            # Telescoping form: c[idx] = c0 + sum_{k=1..15} (idx>=k)*(ck-ck-1)
            # Stock ops only; tiles split between DVE and GPSIMD engines.
            dk = [float(cvals[k] - cvals[k - 1]) for k in range(1, 16)]
            yts = []
            for j in range(nj):
                eng = nc.vector
                enm = "v"
                idx_t = idxp.tile([P, osh], bf16, name="idx_t", tag=f"idx{enm}")
                nc.sync.dma_start(idx_t[:], idxT_d[j * P:(j + 1) * P, :])
                acc = idxp.tile([P, osh], f32, name="acc", tag=f"acc{enm}",
                                bufs=2)
                eng.tensor_scalar(acc[:], idx_t[:], 1.0, dk[0],
                                  mybir.AluOpType.is_ge, mybir.AluOpType.mult)
                tmp = idxp.tile([P, osh], f32, name="tmp", tag=f"tmp{enm}",
                                bufs=2)
                for k in range(2, 16):
                    eng.tensor_scalar(tmp[:], idx_t[:], float(k), dk[k - 1],
                                      mybir.AluOpType.is_ge,
                                      mybir.AluOpType.mult)
                    eng.tensor_tensor(acc[:], acc[:], tmp[:],
                                      mybir.AluOpType.add)
                y_t = ytsp.tile([P, osh], bf16, name="y_t", tag=f"yts{j}")
                eng.tensor_scalar(y_t[:], acc[:], float(cvals[0]), None,
                                  mybir.AluOpType.add)
                yts.append(y_t)

            # ---- Stage 2: wt[i_blk][p_i, o] = sum_j Pi[j, i] * yts[j, o] ---
            wts = []
            for ig in range(ngrp):
                pi_ts = []
                for k in range(igrp):
                    i_blk = ig * igrp + k
                    pi_t = pip.tile([P, nj, P], bf16, name="pi_t", tag="pi")
                    nc.sync.dma_start(
                        pi_t[:], piR_d[i_blk].rearrange("(a p) i -> p a i", p=P)
                    )
                    pi_ts.append(pi_t)
                ps = [
                    wpsum.tile([P, osh], f32, name="wps", tag=f"wps{k}")
                    for k in range(igrp)
                ]
                for j in range(nj):
                    for k in range(igrp):
                        nc.tensor.matmul(
                            ps[k][:], pi_ts[k][:, j, :], yts[j][:],
                            start=(j == 0), stop=(j == nj - 1),
                        )
                for k in range(igrp):
                    i_blk = ig * igrp + k
                    wt_t = wtp.tile([P, osh], bf16, name="wt_t", tag=f"wt{i_blk}")
                    nc.scalar.copy(wt_t[:], ps[k][:])
                    wts.append(wt_t)

            # ---- Stage 3: outT[o, t] = rn[o] * sum_i wt[i,o]*xT[i,t] + b[o]
            for tb in range(nt):
                xt_t = xtp.tile([P, ni, tch], bf16, name="xt_t", tag="xt")
                nc.sync.dma_start(
                    xt_t[:],
                    xT_d[:, tb * tch:(tb + 1) * tch].rearrange(
                        "(a p) t -> p a t", p=P
                    ),
                )
                for ob in range(nob):
                    mp = mpsum.tile([P, tch], f32, name="mp", tag="mp")
                    for i_blk in range(ni):
                        nc.tensor.matmul(
                            mp[:], wts[i_blk][:, ob * P:(ob + 1) * P],
                            xt_t[:, i_blk, :],
                            start=(i_blk == 0), stop=(i_blk == ni - 1),
                        )
                    o_t = outp.tile([P, tch], f32, name="o_t", tag="out")
                    nc.vector.tensor_scalar(
                        o_t[:], mp[:], rn_sb[:, ob:ob + 1], bias_sb[:, ob:ob + 1],
                        mybir.AluOpType.mult, mybir.AluOpType.add,
                    )
                    nc.scalar.dma_start(
                        outT_d[ob * P:(ob + 1) * P, tb * tch:(tb + 1) * tch],
                        o_t[:],
                    )
    nc.compile()
    return nc


def _prep_inputs(x, indices, Pi, row_norms, bias):
    """Host-side layout prep + sharding. Returns list of per-core in_maps."""
    import ml_dtypes

    bf16 = ml_dtypes.bfloat16
    x2 = np.ascontiguousarray(
        np.asarray(x, np.float32).reshape(T, IN).T
    ).astype(bf16)  # (IN, T)
    ni = IN // P
    piR = np.ascontiguousarray(
        np.asarray(Pi, np.float32).astype(bf16).reshape(IN, ni, P).transpose(1, 0, 2)
    )  # (ni, IN_j, P_i)
    idxT = np.ascontiguousarray(np.asarray(indices).T).astype(bf16)  # (IN, OUT)
    rn = np.asarray(row_norms, np.float32)
    bs = np.asarray(bias, np.float32)

    osh = OUT // NCORES
    in_maps = []
    for c in range(NCORES):
        sl = slice(c * osh, (c + 1) * osh)
        in_maps.append({
            "xT": x2,
            "PiR": piR,
            "idxT": np.ascontiguousarray(idxT[:, sl]),
            "rn": np.ascontiguousarray(rn[sl]),
            "bias": np.ascontiguousarray(bs[sl]),
        })
    return in_maps


def _get_nc(centroids):
    key = np.asarray(centroids, np.float32).tobytes()
    nc = _NC_CACHE.get(key)
    if nc is None:
        cvals = [float(v) for v in np.asarray(centroids, np.float32)]
        assert len(cvals) == 16
        nc = build_nc(cvals)
        _NC_CACHE.clear()
        _NC_CACHE[key] = nc
    return nc


def kernel(x, indices, centroids, Pi, row_norms, bias):
    from concourse.bass_utils import run_bass_kernel_spmd

    nc = _get_nc(centroids)
    in_maps = _prep_inputs(x, indices, Pi, row_norms, bias)
    res = run_bass_kernel_spmd(nc, in_maps, list(range(NCORES)))
    shards = [np.asarray(res.results[c]["outT"]) for c in range(NCORES)]
    full = np.concatenate(shards, axis=0)           # (OUT, T)
    out = np.ascontiguousarray(full.T).reshape(B, S, OUT)
    return out.astype(np.float32)



# revision 3
# speedup vs baseline: 35679.8693x; 35679.8693x over previous
"""Trainium2 Bass kernel for CompressedLinear (VQ codebook linear layer).

Computes: out = x @ W^T + bias, where
  W = (centroids[indices] @ Pi) * row_norms[:, None]

Sharding: out_features (4096) split across 8 cores (512 each); x/Pi replicated.
Per-core device pipeline (all DMAs are large contiguous-per-partition-line
transfers; the host pre-permutes every tensor into the exact SBUF layout):
  1. Gather y^T[j,o] = centroids[idxT[j,o]] on DVE (custom fused ops: 2
     codebook entries per instruction; stock telescoping fallback).
  2. W^T[i,o] = sum_j Pi[j,i] * y^T[j,o] on PE (bf16, f32 psum), streamed
     over 16 i-column groups; W^T stays resident in SBUF (bf16).
  3. out^T-ish: for each 512-token chunk: psum[o_p,t] = sum_i W^T x^T;
     DVE applies rn*acc+bias; contiguous 1MB store per chunk.
Host reassembles the 8 per-core [nt, 128, nob, tch] outputs.
"""

import numpy as np

# Problem geometry (hardcoded per contract)
OUT, IN = 4096, 4096
B, S = 4, 2048
T = B * S            # 8192 tokens
NCORES = 8
P = 128              # partitions
OSH = OUT // NCORES  # 512 out features per core
NJ = IN // P         # 32 j blocks (rows of Pi / contraction of stage 2)
NI = IN // P         # 32 i blocks (cols of Pi / contraction of stage 3)
NOB = OSH // P       # 4 output-feature blocks per core
TCH = 512            # token chunk
NT = T // TCH        # 16
GSZ = 256            # i columns per stage-2 group
NG = IN // GSZ       # 16 groups
JC = 8               # j blocks per idx DMA chunk
NJC = NJ // JC       # 4 chunks
JSUB = 2             # j blocks per gather DVE slice

GATHER_MODE = "custom"   # "custom" | "stock"

_DVE_OPS = None
_NC_CACHE = {}


def _register_dve_ops():
    """Register the two fused VQ-gather ops in dve_ops.OPS (idempotent)."""
    global _DVE_OPS
    if _DVE_OPS is not None:
        return _DVE_OPS
    import concourse.dve_ops as dvo
    from concourse.dve_spec import Spec, Src0, Src1, C0, C1, C2, One, eq, lower
    from concourse.dve_uop import DveOpSpec

    existing = {op.name: op for op in dvo.OPS}
    if "VQ_PAIR" in existing:
        _DVE_OPS = {k: existing[k] for k in ("VQ_PAIR", "VQ_ACC2")}
        return _DVE_OPS

    ver = "v3"  # TRN2

    def mk(name, spec, rd1):
        opcode = dvo._CUSTOM_DVE_ROW_BASE + len(dvo.OPS)
        dvo._SUB_OPCODE_FOR_NAME[name] = opcode
        s = DveOpSpec(name=name, opcode=opcode, uops=lower(spec, ver=ver), rd1_en=rd1)
        op = dvo.DveOp(name, spec, subdim=False, uops_sha={ver: s.sha(ver)})
        dvo.OPS.append(op)
        dvo.CUSTOM_DVE_SPECS[name] = spec
        return op

    # out = (idx==imm2)*s0 + (idx==imm2+1)*s1
    pair = mk(
        "VQ_PAIR",
        Spec(
            body=eq(Src0, C2) * C0 + eq(Src0, C2 + One) * C1,
            reference=lambda in0, in1, s0, s1, imm2: (
                (in0 == imm2) * s0 + (in0 == imm2 + 1) * s1
            ).astype(np.float32),
        ),
        False,
    )
    # out = acc + (idx==imm2)*s0 + (idx==imm2+1)*s1
    acc = mk(
        "VQ_ACC2",
        Spec(
            body=Src1 + eq(Src0, C2) * C0 + eq(Src0, C2 + One) * C1,
            reference=lambda in0, in1, s0, s1, imm2: (
                in1 + (in0 == imm2) * s0 + (in0 == imm2 + 1) * s1
            ).astype(np.float32),
        ),
        True,
    )
    _DVE_OPS = {"VQ_PAIR": pair, "VQ_ACC2": acc}
    return _DVE_OPS


def build_nc(cvals, gather_mode=GATHER_MODE, repeat=1,
             wps_bufs=2, mps_bufs=4, gp_slices=0):
    """Build the SPMD Bass program. cvals: 16 python floats (codebook).

    repeat>1 re-runs the whole pipeline that many times inside one NEFF
    (timing amplification only; output is identical).
    gp_slices: number of gather slices (of 16) offloaded to GPSIMD
    (stock telescoping) to run concurrently with the DVE custom-op chain."""
    import concourse.bacc as bacc
    import concourse.mybir as mybir
    from concourse.tile import TileContext

    f32 = mybir.dt.float32
    bf16 = mybir.dt.bfloat16

    nc = bacc.Bacc()
    idx_d = nc.dram_tensor("idxc", [NJC, P, JC, OSH], bf16, kind="ExternalInput")
    pi_d = nc.dram_tensor("pig", [NG, P, NJ, GSZ], bf16, kind="ExternalInput")
    x_d = nc.dram_tensor("xc", [NT, P, NI, TCH], bf16, kind="ExternalInput")
    rn_d = nc.dram_tensor("rn", [P, NOB], f32, kind="ExternalInput")
    bias_d = nc.dram_tensor("bias", [P, NOB], f32, kind="ExternalInput")
    out_d = nc.dram_tensor("outc", [NT, P, NOB, TCH], f32, kind="ExternalOutput")

    if gather_mode == "custom":
        ops = _register_dve_ops()

    with TileContext(nc) as tc:
        with (
            tc.tile_pool(name="constp", bufs=1) as constp,
            tc.tile_pool(name="idxp", bufs=2) as idxp,
            tc.tile_pool(name="accp", bufs=1) as accp,
            tc.tile_pool(name="ytsp", bufs=1) as ytsp,
            tc.tile_pool(name="pip", bufs=2) as pip,
            tc.tile_pool(name="wtp", bufs=1) as wtp,
            tc.tile_pool(name="xtp", bufs=2) as xtp,
            tc.tile_pool(name="outp", bufs=2) as outp,
            tc.tile_pool(name="wpsum", bufs=wps_bufs, space="PSUM") as wpsum,
            tc.tile_pool(name="mpsum", bufs=mps_bufs, space="PSUM") as mpsum,
        ):
          for _rep in range(repeat):
            rn_sb = constp.tile([P, NOB], f32, name="rn_sb", tag="rn_sb")
            nc.sync.dma_start(rn_sb[:], rn_d[:])
            bias_sb = constp.tile([P, NOB], f32, name="bias_sb", tag="bias_sb")
            nc.sync.dma_start(bias_sb[:], bias_d[:])

            # ---- Stage 1: codebook gather: yts[p, j_blk, o] ------------
            yts = ytsp.tile([P, NJ, OSH], bf16, name="yts")
            NSL = JSUB * OSH  # elements per gather slice
            NSLICE = NJ // JSUB
            gp_set = set()
            if gp_slices:
                # spread offloaded slices evenly (one per chunk tail first)
                per_chunk = JC // JSUB
                k = 0
                for r in range(per_chunk - 1, -1, -1):
                    for c in range(NJC):
                        if k < gp_slices:
                            gp_set.add(c * per_chunk + r)
                            k += 1
            dk = [float(cvals[k] - cvals[k - 1]) for k in range(1, 16)]
            for c in range(NJC):
                idx_t = idxp.tile([P, JC, OSH], bf16, name="idx_t", tag="idx")
                nc.sync.dma_start(idx_t[:], idx_d[c])
                for s in range(JC // JSUB):
                    j0 = c * JC + s * JSUB
                    sg = c * (JC // JSUB) + s
                    src = idx_t[:, s * JSUB:(s + 1) * JSUB, :]
                    dst = yts[:, j0:j0 + JSUB, :]
                    if sg in gp_set:
                        eng = nc.gpsimd
                        acc = accp.tile([P, NSL], bf16, name="gacc0", tag="gacc0")
                        tmp = accp.tile([P, NSL], bf16, name="gacc1", tag="gacc1")
                        eng.tensor_scalar(
                            acc[:], src, 1.0, dk[0],
                            mybir.AluOpType.is_ge, mybir.AluOpType.mult)
                        for k in range(2, 16):
                            eng.tensor_scalar(
                                tmp[:], src, float(k), dk[k - 1],
                                mybir.AluOpType.is_ge, mybir.AluOpType.mult)
                            eng.tensor_tensor(
                                acc[:], acc[:], tmp[:], mybir.AluOpType.add)
                        eng.tensor_scalar(
                            dst, acc[:], float(cvals[0]), None,
                            mybir.AluOpType.add)
                    elif gather_mode == "custom":
                        a0 = accp.tile([P, NSL], f32, name="acc0", tag="acc0")
                        a1 = accp.tile([P, NSL], f32, name="acc1", tag="acc1")
                        nc.vector._custom_dve(
                            ops["VQ_PAIR"], out=a0[:], in0=src,
                            s0=float(cvals[0]), s1=float(cvals[1]), imm2=0.0)
                        cur, nxt = a0, a1
                        for k in range(1, 8):
                            o = dst if k == 7 else nxt[:]
                            nc.vector._custom_dve(
                                ops["VQ_ACC2"], out=o, in0=src, in1=cur[:],
                                s0=float(cvals[2 * k]), s1=float(cvals[2 * k + 1]),
                                imm2=float(2 * k))
                            cur, nxt = nxt, cur
                    else:
                        acc = accp.tile([P, NSL], f32, name="acc0", tag="acc0")
                        tmp = accp.tile([P, NSL], f32, name="acc1", tag="acc1")
                        nc.vector.tensor_scalar(
                            acc[:], src, 1.0, dk[0],
                            mybir.AluOpType.is_ge, mybir.AluOpType.mult)
                        for k in range(2, 16):
                            nc.vector.tensor_scalar(
                                tmp[:], src, float(k), dk[k - 1],
                                mybir.AluOpType.is_ge, mybir.AluOpType.mult)
                            nc.vector.tensor_tensor(
                                acc[:], acc[:], tmp[:], mybir.AluOpType.add)
                        nc.vector.tensor_scalar(
                            dst, acc[:], float(cvals[0]), None,
                            mybir.AluOpType.add)

            # ---- Stage 2: wt[i_blk][p_i, o] = sum_j Pi[j, i] * yts[j, o]
            KPG = GSZ // P  # i blocks per group (2)
            wts = []
            for g in range(NG):
                pi_t = pip.tile([P, NJ, GSZ], bf16, name="pi_t", tag="pi")
                nc.sync.dma_start(pi_t[:], pi_d[g])
                ps = [
                    wpsum.tile([P, OSH], f32, name="wps", tag=f"wps{k}")
                    for k in range(KPG)
                ]
                for j in range(NJ):
                    for k in range(KPG):
                        nc.tensor.matmul(
                            ps[k][:], pi_t[:, j, k * P:(k + 1) * P],
                            yts[:, j, :],
                            start=(j == 0), stop=(j == NJ - 1),
                        )
                for k in range(KPG):
                    i_blk = g * KPG + k
                    wt_t = wtp.tile([P, OSH], bf16, name="wt_t", tag=f"wt{i_blk}")
                    nc.scalar.copy(wt_t[:], ps[k][:])
                    wts.append(wt_t)

            # ---- Stage 3: out[o_p, t] = rn[o] * sum_i wt[i,o]*xT[i,t] + b[o]
            for tb in range(NT):
                xt_t = xtp.tile([P, NI, TCH], bf16, name="xt_t", tag="xt")
                nc.sync.dma_start(xt_t[:], x_d[tb])
                o_all = outp.tile([P, NOB, TCH], f32, name="o_all", tag="out")
                for ob in range(NOB):
                    mp = mpsum.tile([P, TCH], f32, name="mp", tag="mp")
                    for i_blk in range(NI):
                        nc.tensor.matmul(
                            mp[:], wts[i_blk][:, ob * P:(ob + 1) * P],
                            xt_t[:, i_blk, :],
                            start=(i_blk == 0), stop=(i_blk == NI - 1),
                        )
                    nc.vector.tensor_scalar(
                        o_all[:, ob, :], mp[:], rn_sb[:, ob:ob + 1],
                        bias_sb[:, ob:ob + 1],
                        mybir.AluOpType.mult, mybir.AluOpType.add,
                    )
                nc.scalar.dma_start(out_d[tb], o_all[:])
    nc.compile()
    return nc


def _prep_inputs(x, indices, Pi, row_norms, bias):
    """Host-side layout prep + sharding. Returns list of per-core in_maps."""
    import ml_dtypes

    bf16 = ml_dtypes.bfloat16

    # x: [NT, P, NI, TCH]; xh[tb, p, i, t'] = x[tb*TCH+t', i*P+p]
    xf = np.asarray(x, np.float32).reshape(T, IN)
    xh = np.ascontiguousarray(
        xf.reshape(NT, TCH, NI, P).transpose(0, 3, 2, 1)
    ).astype(bf16)

    # Pi: [NG, P, NJ, GSZ]; pih[g, p, j, i'] = Pi[j*P+p, g*GSZ+i']
    pih = np.ascontiguousarray(
        np.asarray(Pi, np.float32).astype(bf16)
        .reshape(NJ, P, NG, GSZ).transpose(2, 1, 0, 3)
    )

    # idx per core: [NJC, P, JC, OSH]; ih[c, p, jj, o] = idxT[(c*JC+jj)*P+p, o]
    idxT = np.asarray(indices).T  # [IN(j), OUT(o)]
    rn = np.asarray(row_norms, np.float32)
    bs = np.asarray(bias, np.float32)

    in_maps = []
    for c in range(NCORES):
        sl = slice(c * OSH, (c + 1) * OSH)
        ih = np.ascontiguousarray(
            idxT[:, sl].reshape(NJC, JC, P, OSH).transpose(0, 2, 1, 3)
        ).astype(bf16)
        in_maps.append({
            "idxc": ih,
            "pig": pih,
            "xc": xh,
            "rn": np.ascontiguousarray(rn[sl].reshape(NOB, P).T),
            "bias": np.ascontiguousarray(bs[sl].reshape(NOB, P).T),
        })
    return in_maps


def _get_nc(centroids):
    key = np.asarray(centroids, np.float32).tobytes()
    nc = _NC_CACHE.get(key)
    if nc is None:
        cvals = [float(v) for v in np.asarray(centroids, np.float32)]
        assert len(cvals) == 16
        nc = build_nc(cvals)
        _NC_CACHE.clear()
        _NC_CACHE[key] = nc
    return nc


def _unshard(shards):
    """shards: 8 arrays [NT, P, NOB, TCH] -> (B, S, OUT) f32."""
    cols = [
        np.asarray(sh).transpose(0, 3, 2, 1).reshape(T, OSH)
        for sh in shards
    ]
    return np.ascontiguousarray(
        np.concatenate(cols, axis=1)
    ).reshape(B, S, OUT).astype(np.float32)


def kernel(x, indices, centroids, Pi, row_norms, bias):
    from concourse.bass_utils import run_bass_kernel_spmd

    nc = _get_nc(centroids)
    in_maps = _prep_inputs(x, indices, Pi, row_norms, bias)
    res = run_bass_kernel_spmd(nc, in_maps, list(range(NCORES)))
    return _unshard([res.results[c]["outc"] for c in range(NCORES)])


# revision 4
# speedup vs baseline: 35839.7653x; 1.0045x over previous
"""Trainium2 Bass kernel for CompressedLinear (VQ codebook linear layer).

Computes: out = x @ W^T + bias, where
  W = (centroids[indices] @ Pi) * row_norms[:, None]

Sharding: out_features (4096) split across 8 cores (512 each); x/Pi replicated.
Per-core device pipeline (all DMAs are large contiguous-per-partition-line
transfers; the host pre-permutes every tensor into the exact SBUF layout):
  1. Gather y^T[j,o] = centroids[idxT[j,o]] on DVE (custom fused ops: 2
     codebook entries per instruction; stock telescoping fallback).
  2. W^T[i,o] = sum_j Pi[j,i] * y^T[j,o] on PE (bf16, f32 psum), streamed
     over 16 i-column groups; W^T stays resident in SBUF (bf16).
  3. out^T-ish: for each 512-token chunk: psum[o_p,t] = sum_i W^T x^T;
     DVE applies rn*acc+bias; contiguous 1MB store per chunk.
Host reassembles the 8 per-core [nt, 128, nob, tch] outputs.
"""

import numpy as np

# Problem geometry (hardcoded per contract)
OUT, IN = 4096, 4096
B, S = 4, 2048
T = B * S            # 8192 tokens
NCORES = 8
P = 128              # partitions
OSH = OUT // NCORES  # 512 out features per core
NJ = IN // P         # 32 j blocks (rows of Pi / contraction of stage 2)
NI = IN // P         # 32 i blocks (cols of Pi / contraction of stage 3)
NOB = OSH // P       # 4 output-feature blocks per core
TCH = 512            # token chunk
NT = T // TCH        # 16
GSZ = 256            # i columns per stage-2 group
NG = IN // GSZ       # 16 groups
JC = 4               # j blocks per idx DMA chunk
NJC = NJ // JC       # 8 chunks
JSUB = 2             # j blocks per gather DVE slice

GATHER_MODE = "custom"   # "custom" | "stock"

_DVE_OPS = None
_NC_CACHE = {}


def _register_dve_ops():
    """Register the two fused VQ-gather ops in dve_ops.OPS (idempotent)."""
    global _DVE_OPS
    if _DVE_OPS is not None:
        return _DVE_OPS
    import concourse.dve_ops as dvo
    from concourse.dve_spec import Spec, Src0, Src1, C0, C1, C2, One, eq, lower
    from concourse.dve_uop import DveOpSpec

    existing = {op.name: op for op in dvo.OPS}
    if "VQ_PAIR" in existing:
        _DVE_OPS = {k: existing[k] for k in ("VQ_PAIR", "VQ_ACC2")}
        return _DVE_OPS

    ver = "v3"  # TRN2

    def mk(name, spec, rd1):
        opcode = dvo._CUSTOM_DVE_ROW_BASE + len(dvo.OPS)
        dvo._SUB_OPCODE_FOR_NAME[name] = opcode
        s = DveOpSpec(name=name, opcode=opcode, uops=lower(spec, ver=ver), rd1_en=rd1)
        op = dvo.DveOp(name, spec, subdim=False, uops_sha={ver: s.sha(ver)})
        dvo.OPS.append(op)
        dvo.CUSTOM_DVE_SPECS[name] = spec
        return op

    # out = (idx==imm2)*s0 + (idx==imm2+1)*s1
    pair = mk(
        "VQ_PAIR",
        Spec(
            body=eq(Src0, C2) * C0 + eq(Src0, C2 + One) * C1,
            reference=lambda in0, in1, s0, s1, imm2: (
                (in0 == imm2) * s0 + (in0 == imm2 + 1) * s1
            ).astype(np.float32),
        ),
        False,
    )
    # out = acc + (idx==imm2)*s0 + (idx==imm2+1)*s1
    acc = mk(
        "VQ_ACC2",
        Spec(
            body=Src1 + eq(Src0, C2) * C0 + eq(Src0, C2 + One) * C1,
            reference=lambda in0, in1, s0, s1, imm2: (
                in1 + (in0 == imm2) * s0 + (in0 == imm2 + 1) * s1
            ).astype(np.float32),
        ),
        True,
    )
    _DVE_OPS = {"VQ_PAIR": pair, "VQ_ACC2": acc}
    return _DVE_OPS


def build_nc(cvals, gather_mode=GATHER_MODE, repeat=1,
             wps_bufs=2, mps_bufs=4, gp_slices=0):
    """Build the SPMD Bass program. cvals: 16 python floats (codebook).

    repeat>1 re-runs the whole pipeline that many times inside one NEFF
    (timing amplification only; output is identical).
    gp_slices: number of gather slices (of 16) offloaded to GPSIMD
    (stock telescoping) to run concurrently with the DVE custom-op chain."""
    import concourse.bacc as bacc
    import concourse.mybir as mybir
    from concourse.tile import TileContext

    f32 = mybir.dt.float32
    bf16 = mybir.dt.bfloat16

    nc = bacc.Bacc()
    idx_d = nc.dram_tensor("idxc", [NJC, P, JC, OSH], bf16, kind="ExternalInput")
    pi_d = nc.dram_tensor("pig", [NG, P, NJ, GSZ], bf16, kind="ExternalInput")
    x_d = nc.dram_tensor("xc", [NT, P, NI, TCH], bf16, kind="ExternalInput")
    rn_d = nc.dram_tensor("rn", [P, NOB], f32, kind="ExternalInput")
    bias_d = nc.dram_tensor("bias", [P, NOB], f32, kind="ExternalInput")
    out_d = nc.dram_tensor("outc", [NT, P, NOB, TCH], f32, kind="ExternalOutput")

    if gather_mode == "custom":
        ops = _register_dve_ops()

    with TileContext(nc) as tc:
        with (
            tc.tile_pool(name="constp", bufs=1) as constp,
            tc.tile_pool(name="idxp", bufs=2) as idxp,
            tc.tile_pool(name="accp", bufs=1) as accp,
            tc.tile_pool(name="ytsp", bufs=1) as ytsp,
            tc.tile_pool(name="pip", bufs=2) as pip,
            tc.tile_pool(name="wtp", bufs=1) as wtp,
            tc.tile_pool(name="xtp", bufs=2) as xtp,
            tc.tile_pool(name="outp", bufs=2) as outp,
            tc.tile_pool(name="wpsum", bufs=wps_bufs, space="PSUM") as wpsum,
            tc.tile_pool(name="mpsum", bufs=mps_bufs, space="PSUM") as mpsum,
        ):
          for _rep in range(repeat):
            rn_sb = constp.tile([P, NOB], f32, name="rn_sb", tag="rn_sb")
            nc.sync.dma_start(rn_sb[:], rn_d[:])
            bias_sb = constp.tile([P, NOB], f32, name="bias_sb", tag="bias_sb")
            nc.sync.dma_start(bias_sb[:], bias_d[:])

            # ---- Stage 1: codebook gather: yts[p, j_blk, o] ------------
            yts = ytsp.tile([P, NJ, OSH], bf16, name="yts")
            NSL = JSUB * OSH  # elements per gather slice
            NSLICE = NJ // JSUB
            gp_set = set()
            if gp_slices:
                # spread offloaded slices evenly (one per chunk tail first)
                per_chunk = JC // JSUB
                k = 0
                for r in range(per_chunk - 1, -1, -1):
                    for c in range(NJC):
                        if k < gp_slices:
                            gp_set.add(c * per_chunk + r)
                            k += 1
            dk = [float(cvals[k] - cvals[k - 1]) for k in range(1, 16)]
            for c in range(NJC):
                idx_t = idxp.tile([P, JC, OSH], bf16, name="idx_t", tag="idx")
                nc.sync.dma_start(idx_t[:], idx_d[c])
                for s in range(JC // JSUB):
                    j0 = c * JC + s * JSUB
                    sg = c * (JC // JSUB) + s
                    src = idx_t[:, s * JSUB:(s + 1) * JSUB, :]
                    dst = yts[:, j0:j0 + JSUB, :]
                    if sg in gp_set:
                        eng = nc.gpsimd
                        acc = accp.tile([P, NSL], bf16, name="gacc0", tag="gacc0")
                        tmp = accp.tile([P, NSL], bf16, name="gacc1", tag="gacc1")
                        eng.tensor_scalar(
                            acc[:], src, 1.0, dk[0],
                            mybir.AluOpType.is_ge, mybir.AluOpType.mult)
                        for k in range(2, 16):
                            eng.tensor_scalar(
                                tmp[:], src, float(k), dk[k - 1],
                                mybir.AluOpType.is_ge, mybir.AluOpType.mult)
                            eng.tensor_tensor(
                                acc[:], acc[:], tmp[:], mybir.AluOpType.add)
                        eng.tensor_scalar(
                            dst, acc[:], float(cvals[0]), None,
                            mybir.AluOpType.add)
                    elif gather_mode == "custom":
                        a0 = accp.tile([P, NSL], f32, name="acc0", tag="acc0")
                        a1 = accp.tile([P, NSL], f32, name="acc1", tag="acc1")
                        nc.vector._custom_dve(
                            ops["VQ_PAIR"], out=a0[:], in0=src,
                            s0=float(cvals[0]), s1=float(cvals[1]), imm2=0.0)
                        cur, nxt = a0, a1
                        for k in range(1, 8):
                            o = dst if k == 7 else nxt[:]
                            nc.vector._custom_dve(
                                ops["VQ_ACC2"], out=o, in0=src, in1=cur[:],
                                s0=float(cvals[2 * k]), s1=float(cvals[2 * k + 1]),
                                imm2=float(2 * k))
                            cur, nxt = nxt, cur
                    else:
                        acc = accp.tile([P, NSL], f32, name="acc0", tag="acc0")
                        tmp = accp.tile([P, NSL], f32, name="acc1", tag="acc1")
                        nc.vector.tensor_scalar(
                            acc[:], src, 1.0, dk[0],
                            mybir.AluOpType.is_ge, mybir.AluOpType.mult)
                        for k in range(2, 16):
                            nc.vector.tensor_scalar(
                                tmp[:], src, float(k), dk[k - 1],
                                mybir.AluOpType.is_ge, mybir.AluOpType.mult)
                            nc.vector.tensor_tensor(
                                acc[:], acc[:], tmp[:], mybir.AluOpType.add)
                        nc.vector.tensor_scalar(
                            dst, acc[:], float(cvals[0]), None,
                            mybir.AluOpType.add)

            # ---- Stage 2: wt[i_blk][p_i, o] = sum_j Pi[j, i] * yts[j, o]
            KPG = GSZ // P  # i blocks per group (2)
            wts = []
            for g in range(NG):
                pi_t = pip.tile([P, NJ, GSZ], bf16, name="pi_t", tag="pi")
                nc.sync.dma_start(pi_t[:], pi_d[g])
                ps = [
                    wpsum.tile([P, OSH], f32, name="wps", tag=f"wps{k}")
                    for k in range(KPG)
                ]
                for j in range(NJ):
                    for k in range(KPG):
                        nc.tensor.matmul(
                            ps[k][:], pi_t[:, j, k * P:(k + 1) * P],
                            yts[:, j, :],
                            start=(j == 0), stop=(j == NJ - 1),
                        )
                for k in range(KPG):
                    i_blk = g * KPG + k
                    wt_t = wtp.tile([P, OSH], bf16, name="wt_t", tag=f"wt{i_blk}")
                    nc.scalar.copy(wt_t[:], ps[k][:])
                    wts.append(wt_t)

            # ---- Stage 3: out[o_p, t] = rn[o] * sum_i wt[i,o]*xT[i,t] + b[o]
            for tb in range(NT):
                xt_t = xtp.tile([P, NI, TCH], bf16, name="xt_t", tag="xt")
                nc.sync.dma_start(xt_t[:], x_d[tb])
                o_all = outp.tile([P, NOB, TCH], f32, name="o_all", tag="out")
                for ob in range(NOB):
                    mp = mpsum.tile([P, TCH], f32, name="mp", tag="mp")
                    for i_blk in range(NI):
                        nc.tensor.matmul(
                            mp[:], wts[i_blk][:, ob * P:(ob + 1) * P],
                            xt_t[:, i_blk, :],
                            start=(i_blk == 0), stop=(i_blk == NI - 1),
                        )
                    nc.vector.tensor_scalar(
                        o_all[:, ob, :], mp[:], rn_sb[:, ob:ob + 1],
                        bias_sb[:, ob:ob + 1],
                        mybir.AluOpType.mult, mybir.AluOpType.add,
                    )
                for ob in range(NOB):
                    nc.scalar.dma_start(out_d[tb, :, ob, :], o_all[:, ob, :])
    nc.compile()
    return nc


def _prep_inputs(x, indices, Pi, row_norms, bias):
    """Host-side layout prep + sharding. Returns list of per-core in_maps."""
    import ml_dtypes

    bf16 = ml_dtypes.bfloat16

    # x: [NT, P, NI, TCH]; xh[tb, p, i, t'] = x[tb*TCH+t', i*P+p]
    xf = np.asarray(x, np.float32).reshape(T, IN)
    xh = np.ascontiguousarray(
        xf.reshape(NT, TCH, NI, P).transpose(0, 3, 2, 1)
    ).astype(bf16)

    # Pi: [NG, P, NJ, GSZ]; pih[g, p, j, i'] = Pi[j*P+p, g*GSZ+i']
    pih = np.ascontiguousarray(
        np.asarray(Pi, np.float32).astype(bf16)
        .reshape(NJ, P, NG, GSZ).transpose(2, 1, 0, 3)
    )

    # idx per core: [NJC, P, JC, OSH]; ih[c, p, jj, o] = idxT[(c*JC+jj)*P+p, o]
    idxT = np.asarray(indices).T  # [IN(j), OUT(o)]
    rn = np.asarray(row_norms, np.float32)
    bs = np.asarray(bias, np.float32)

    in_maps = []
    for c in range(NCORES):
        sl = slice(c * OSH, (c + 1) * OSH)
        ih = np.ascontiguousarray(
            idxT[:, sl].reshape(NJC, JC, P, OSH).transpose(0, 2, 1, 3)
        ).astype(bf16)
        in_maps.append({
            "idxc": ih,
            "pig": pih,
            "xc": xh,
            "rn": np.ascontiguousarray(rn[sl].reshape(NOB, P).T),
            "bias": np.ascontiguousarray(bs[sl].reshape(NOB, P).T),
        })
    return in_maps


def _get_nc(centroids):
    key = np.asarray(centroids, np.float32).tobytes()
    nc = _NC_CACHE.get(key)
    if nc is None:
        cvals = [float(v) for v in np.asarray(centroids, np.float32)]
        assert len(cvals) == 16
        nc = build_nc(cvals)
        _NC_CACHE.clear()
        _NC_CACHE[key] = nc
    return nc


def _unshard(shards):
    """shards: 8 arrays [NT, P, NOB, TCH] -> (B, S, OUT) f32."""
    cols = [
        np.asarray(sh).transpose(0, 3, 2, 1).reshape(T, OSH)
        for sh in shards
    ]
    return np.ascontiguousarray(
        np.concatenate(cols, axis=1)
    ).reshape(B, S, OUT).astype(np.float32)


def kernel(x, indices, centroids, Pi, row_norms, bias):
    from concourse.bass_utils import run_bass_kernel_spmd

    nc = _get_nc(centroids)
    in_maps = _prep_inputs(x, indices, Pi, row_norms, bias)
    res = run_bass_kernel_spmd(nc, in_maps, list(range(NCORES)))
    return _unshard([res.results[c]["outc"] for c in range(NCORES)])
